# revision 24
# baseline (speedup 1.0000x reference)
"""DILATE loss (soft-DTW shape + temporal distortion) Trainium2 Bass kernel.

Math (per batch element, N=256, gamma=0.01, alpha=0.8):
  D[i,j] = (t_i - p_j)^2
  soft-DTW DP: R[i,j] = D[i,j] + softmin_g(R[i-1,j-1], R[i-1,j], R[i,j-1])
  loss = alpha*mean_b R[N,N] + (1-alpha)*sum_ij mean_b(E)*(i-j)^2 / N^2,
  E = dR[N,N]/dD.

Kernel strategy:
  * gamma is tiny, so the hard-min DP is within ~5e-4 of the soft DP; each
    DP row is ONE raw tensor_tensor_scan(min,add) with interleaved APs
    (2 stream elements per cell: e1 mins the diagonal pred, e2 mins the
    up pred and adds D_j, written compactly via a step-0 output dim), with
    the D row produced on the scalar engine via Square(p + bias=-t_i).
    Forward and reverse DPs run together on 32 partitions per core.
  * E uses the forward/backward identity
      E[i,j] = exp((R[N,N] - Rf[i,j] - Rb[i,j] + D[i,j])/gamma)
    (Rb = DP of the axis-reversed cost matrix), fully elementwise.
  * first-order softness correction for the value:
      val_soft ~= val_hard - gamma * sum_ij E[i,j]*ln S[i,j],
      S[i,j] = sum_preds exp((minh[i,j] - Rh[pred])/gamma),
    which cuts total error another ~10x (to ~6e-5 relative).

Distribution: batch 128 -> 16 per core x 8 cores (data parallel; the
sharding_hint's all-reduce is replaced by a host-side combine of tiny
per-core partial sums).

I/O design (dominates wall time on this axon tunnel, where each call costs
~1 network round trip ~25-30ms plus ~70ms per EXTRA output array and a few
ms per extra input array):
  * ONE f16 input "pt" [16, 512] per core (p | t packed on the free axis),
    upcast to f32 on device (f16 quantization adds ~1e-5 rel err, budget
    is 2e-2).
  * ONE [1, 17] f32 output per core: partition-sums of (vals*8 | acc |
    acc2) computed on device by a ones-vector matmul, so only 68 bytes
    cross the tunnel per core.
  * no zero-init buffers are passed for outputs, and the jit uses the
    fast-dispatch (effect-free C++) path.
"""
import numpy as np
from contextlib import ExitStack

import bass_rust
import concourse.bass as bass
import concourse.mybir as mybir
import concourse.tile as tile

ALPHA = 0.8
GAMMA = 0.01
GINV = 1.0 / GAMMA
BIG = 1e8
B, N, NCORES = 128, 256, 8
BPC = B // NCORES          # 16 batches per core
P = 2 * BPC                # 32 scan partitions (fwd + bwd)
GPB = 128 // BPC           # 8 partition groups per batch in staged layout
RPG = N // GPB             # 32 rows per group
F32 = mybir.dt.float32
AF = mybir.ActivationFunctionType
OP = mybir.AluOpType
W = N + 1                  # row slot width (border col + N values)
# staged fwd region: 33 slots (1 overlap row + 32 rows) x 257 each
FOFF = 0
FSLOT = W
FSIZE = 33 * FSLOT
# staged bwd region: 32 slots x 256, natural element order
BOFF = FSIZE
BSIZE = RPG * N
NCHUNK = 8
SPC = RPG // NCHUNK        # 4 row-slots per E-pass chunk
FE = SPC * N               # 1024 free elems per chunk

_RUNNER = []


def _split_multiwaits(nc, max_waits=1):
    """This walrus build rejects any instruction carrying more than one
    semaphore wait ("Too many sync wait commands" at codegen); move excess
    waits onto preceding same-engine NoOps."""
    cnt = 0
    for f in nc.m.functions:
        for blk in f.blocks:
            newinsts = []
            changed = False
            for inst in blk.instructions:
                si = inst.sync_info
                if si is not None and si.on_wait is not None and len(si.on_wait) > max_waits:
                    waits = list(si.on_wait)
                    excess, keep = waits[:-max_waits], waits[-max_waits:]
                    while excess:
                        chunk, excess = excess[:max_waits], excess[max_waits:]
                        cnt += 1
                        newinsts.append(mybir.InstNoOp(
                            name=f"waitsplit{cnt}", engine=inst.engine,
                            ins=[], outs=[],
                            sync_info=mybir.SyncInfo(on_wait=chunk, on_update=[])))
                        changed = True
                    si.on_wait = keep
                newinsts.append(inst)
            if changed:
                blk.instructions[:] = newinsts


def _build_module():
    nc = bass.Bass()
    # single packed f16 input (p in cols 0:N, t in cols N:2N) and single
    # [1,17] output (col 0 = vals sum x GPB, 1:9 = acc col-sums, 9:17 =
    # acc2 col-sums, reduced over partitions on device). Array count and
    # payload bytes dominate per-call cost on this tunnel: each extra
    # output array is a serialized ~70ms round trip, and h2d/d2h move at
    # ~25 MB/s, so f16 input + 68B output shave several ms.
    F16 = mybir.dt.float16
    pt_in = nc.dram_tensor("pt", [BPC, 2 * N], F16, kind="ExternalInput")
    out_t = nc.dram_tensor("out", [1, 1 + 2 * NCHUNK], F32,
                           kind="ExternalOutput")

    with tile.TileContext(nc) as tc, ExitStack() as ctx:
        cpool = ctx.enter_context(tc.tile_pool(name="cpool", bufs=1))
        dpool = ctx.enter_context(tc.tile_pool(name="dpool", bufs=8))
        vpool = ctx.enter_context(tc.tile_pool(name="vpool", bufs=4))
        epool = ctx.enter_context(tc.tile_pool(name="epool", bufs=2))
        spool = ctx.enter_context(tc.tile_pool(name="spool", bufs=1))
        ppool = ctx.enter_context(tc.psum_pool(name="ppool", bufs=1))

        p_buf = cpool.tile([P, N], F32, tag="p_buf")
        t_buf = cpool.tile([P, N], F32, tag="t_buf")
        nt_buf = cpool.tile([P, N], F32, tag="nt_buf")
        tmp = cpool.tile([P, N], F32, tag="tmp")
        tstage = cpool.tile([128, RPG], F32, tag="tstage")
        pstage = cpool.tile([128, N], F32, tag="pstage")
        omega = cpool.tile([128, RPG * N], F32, tag="omega")
        g32 = cpool.tile([128, 1], F32, tag="g32")
        # g32[p] = RPG*(p%GPB): iota the 8 group offsets along the free dim,
        # then scatter to the interleaved partition layout with 8 tiny DMAs
        # (compute ops can't address stride-8 partitions; DMA can).
        giota = cpool.tile([BPC, GPB], F32, tag="giota")
        nc.gpsimd.iota(giota[:], pattern=[[RPG, GPB]], base=0,
                       channel_multiplier=0,
                       allow_small_or_imprecise_dtypes=True)
        g3v = g32.rearrange("(x y) f -> x y f", y=GPB)
        for g in range(GPB):
            nc.sync.dma_start(g3v[:, g, :].squeeze(), giota[:, g:g + 1])
        # on-device input prep: f16 DMA in, upcast to f32, fwd halves
        # straight, bwd halves reversed. compute ops must start at a
        # partition quadrant, so reverse at base 0 and DMA into the
        # upper half.
        pt16 = cpool.tile([BPC, 2 * N], F16, tag="pt16")
        nc.sync.dma_start(pt16[:], pt_in.ap())
        nc.vector.tensor_copy(p_buf[0:BPC, :], pt16[:, 0:N])
        nc.vector.tensor_copy(tmp[0:BPC, :], p_buf[0:BPC, ::-1])
        nc.sync.dma_start(p_buf[BPC:P, :], tmp[0:BPC, :])
        nc.vector.tensor_copy(t_buf[0:BPC, :], pt16[:, N:2 * N])
        nc.vector.tensor_copy(tmp[0:BPC, :], t_buf[0:BPC, ::-1])
        nc.sync.dma_start(t_buf[BPC:P, :], tmp[0:BPC, :])
        nc.vector.tensor_scalar_mul(nt_buf[:], t_buf[:], -1.0)
        # staged-layout replicas of p and t, from the upcast SBUF copies
        ts3 = tstage.rearrange("(x y) f -> x y f", y=GPB)
        ps3 = pstage.rearrange("(x y) f -> x y f", y=GPB)
        for g in range(GPB):
            nc.sync.dma_start(ts3[:, g, :].squeeze(),
                              t_buf[0:BPC, g * RPG:(g + 1) * RPG])
            nc.sync.dma_start(ps3[:, g, :].squeeze(), p_buf[0:BPC, :])
        # Omega[p, r*256+jm1] = ((32*(p%8) + r) - jm1)^2, built on device:
        # iota gives (r - jm1) per partition; Square adds the 32g bias.
        nc.gpsimd.iota(omega[:], pattern=[[1, RPG], [-1, N]], base=0,
                       channel_multiplier=0,
                       allow_small_or_imprecise_dtypes=True)
        nc.scalar.activation(omega[:], omega[:], AF.Square,
                             bias=g32[:], scale=1.0)

        stage = spool.tile([128, FSIZE + BSIZE], F32, tag="stage")
        # fwd region views: [x=16, y=8 groups, slot, elem]
        stF = stage[:, FOFF:FOFF + FSIZE].rearrange(
            "(x y) (s w) -> x y s w", y=GPB, w=FSLOT)
        stB = stage[:, BOFF:BOFF + BSIZE].rearrange(
            "(x y) (s w) -> x y s w", y=GPB, w=N)

        # rolling window: slot 0 = initial row [0, BIG...], 16 working slots
        win = cpool.tile([P, 17 * W], F32, tag="win")
        nc.vector.memset(win[:], BIG)
        nc.vector.memset(win[:, 0:1], 0.0)      # R[0,0] = 0
        winf = win[0:BPC].rearrange("p (s w) -> p s w", w=W)
        winb = win[BPC:P].rearrange("p (s w) -> p s w", w=W)

        # stage the fwd border row (row 0) into group 0's overlap slot
        nc.sync.dma_start(stF[:, 0, 0, :].squeeze(), win[0:BPC, 0:W])

        # pre-zero the 8 drow slots: evens stay 0 (the "+0" scan elements)
        for _z in range(8):
            zt = dpool.tile([P, 2 * N], F32, tag="drow")
            nc.vector.memset(zt[:], 0.0)

        V2 = bass_rust.VecI64Pair

        def _ap3(ap, d1, d2):
            part = tuple(ap.ap[0])
            ap.ap = V2([part, d1, d2])
            return ap

        prev_off = 0
        for i in range(1, N + 1):
            k = 1 + (i - 1) % 16
            off = k * W
            drow = dpool.tile([P, 2 * N], F32, tag="drow")
            nc.scalar.activation(drow[:, 1::2], p_buf[:], AF.Square,
                                 bias=nt_buf[:, i - 1:i], scale=1.0)
            # fused 3-way-min DP row: one scan, 2 stream elements per cell:
            #   e1: state = min(Rprev[j-1], state) + 0
            #   e2: state = min(Rprev[j],   state) + D_j
            d0 = _ap3(win[:, prev_off:prev_off + N], (1, N), (1, 2))
            d1 = _ap3(drow[:, 0:2 * N], (2, N), (1, 2))
            o3 = _ap3(win[:, off + 1:off + 1 + N], (1, N), (0, 2))
            eng = nc.vector
            eng.add_instruction(mybir.InstTensorScalarPtr(
                name=nc.get_next_instruction_name(),
                is_tensor_tensor_scan=True, is_scalar_tensor_tensor=True,
                op0=OP.min, op1=OP.add,
                ins=[eng.lower_ap(d0), eng.lower_ap_or_imm(float(BIG)),
                     eng.lower_ap(d1)],
                outs=[eng.lower_ap(o3)]))
            if i % 8 == 0:
                k0 = k - 7
                g, r = (i - 8) // RPG, (i - 8) % RPG
                # fwd rows staged full-width (border col included) at slots 1+r..
                nc.sync.dma_start(stF[:, g, 1 + r:1 + r + 8, :].squeeze(),
                                  winf[:, k0:k0 + 8, :])
                # bwd row i' lands at the (g,r) of fwd row 257-i', natural order
                gb, rb = (N - i) // RPG, (N - i) % RPG
                bstop = rb - 1 if rb > 0 else None
                nc.sync.dma_start(
                    stB[:, gb, rb + 7:bstop:-1, :].squeeze(),
                    winb[:, k0:k0 + 8, 1:W])
                if i % RPG == 0 and i < N:
                    # row i = 32g+32 is also group g+1's overlap slot 0
                    nc.sync.dma_start(stF[:, g + 1, 0, :].squeeze(),
                                      winf[:, k0 + 7, :])
            prev_off = off

        # per-batch DP value val_b = Rf[N,N]: group 7, slot 32, elem 256
        vcol16 = cpool.tile([BPC, 1], F32, tag="vcol16")
        nc.sync.dma_start(vcol16[:],
                          stF[:, GPB - 1, RPG, FSLOT - 1:FSLOT].squeeze())
        val128 = cpool.tile([128, 1], F32, tag="val128")
        v3 = val128.rearrange("(x y) f -> x y f", y=GPB)
        for g in range(GPB):
            nc.sync.dma_start(v3[:, g, :].squeeze(), vcol16[:])
        val100 = cpool.tile([128, 1], F32, tag="val100")
        nc.vector.tensor_scalar_mul(val100[:], val128[:], GINV)
        packed = cpool.tile([128, 1 + 2 * NCHUNK], F32, tag="packed")
        nc.vector.tensor_copy(packed[:, 0:1], val128[:])

        # E-pass over chunks of SPC row-slots:
        #   dq  = 100*D               (ACT Square(10*(t-p)))
        #   ex  = -100*(Rf+Rb) + dq   -> E = Exp(ex + 100*val)
        #   acc += E*Omega
        #   m100 = 100*Rf - dq (= 100*minh); S = sum_preds exp(m100 - 100*P)
        #   acc2 += E*clamp(ln S, 0, 1.2)
        # all-partition (128) views of the staged regions
        eF = stage[:, FOFF:FOFF + FSIZE].rearrange("p (s w) -> p s w", w=FSLOT)
        eB = stage[:, BOFF:BOFF + BSIZE].rearrange("p (s w) -> p s w", w=N)
        for c in range(NCHUNK):
            s0 = c * SPC                       # first row-slot (0-based row idx)
            rf3 = eF[:, 1 + s0:1 + s0 + SPC, 1:W]
            rb3 = eB[:, s0:s0 + SPC, ::-1]
            pa3 = eF[:, s0:s0 + SPC, 0:N]      # R[i-1, j-1]
            pb3 = eF[:, s0:s0 + SPC, 1:W]      # R[i-1, j]
            pc3 = eF[:, 1 + s0:1 + s0 + SPC, 0:N]  # R[i, j-1]

            def t3(tag):
                tl = epool.tile([128, FE], F32, tag=tag)
                return tl, tl.rearrange("p (s w) -> p s w", w=N)

            s1, s13 = t3("s1")
            nc.vector.tensor_tensor(s13, rf3, rb3, op=OP.add)
            u, u3 = t3("u")
            tb = tstage[:, s0:s0 + SPC].unsqueeze(2).broadcast_to((128, SPC, N))
            pb = pstage.unsqueeze(1).broadcast_to((128, SPC, N))
            nc.vector.tensor_tensor(u3, tb, pb, op=OP.subtract)
            nc.scalar.activation(u[:], u[:], AF.Square,
                                 bias=0.0, scale=GINV ** 0.5)  # u <- 100*D
            nc.vector.scalar_tensor_tensor(s1[:], s1[:], -GINV, u[:],
                                           op0=OP.mult, op1=OP.add)  # s1 <- ex
            nc.scalar.activation(s1[:], s1[:], AF.Exp,
                                 bias=val100[:], scale=1.0)          # s1 <- E
            # correction: m100 = 100*Rf - dq
            m100, m1003 = t3("m100")
            nc.vector.scalar_tensor_tensor(m1003, rf3, GINV, u3,
                                           op0=OP.mult, op1=OP.subtract)
            ea, ea3 = t3("ea")
            arg, arg3 = t3("arg")
            # temporal term: out goes to the (not yet used) arg scratch so E
            # in s1 survives for the correction below
            nc.vector.scalar_tensor_tensor(arg[:], s1[:], 1.0,
                                           omega[:, c * FE:(c + 1) * FE],
                                           op0=OP.mult, op1=OP.mult,
                                           accum_out=packed[:, 1 + c:2 + c])
            nc.vector.scalar_tensor_tensor(arg3, pa3, -GINV, m1003,
                                           op0=OP.mult, op1=OP.add)
            nc.scalar.activation(ea[:], arg[:], AF.Exp)
            eb, eb3 = t3("eb")
            nc.vector.scalar_tensor_tensor(arg3, pb3, -GINV, m1003,
                                           op0=OP.mult, op1=OP.add)
            nc.scalar.activation(eb[:], arg[:], AF.Exp)
            nc.vector.tensor_tensor(ea[:], ea[:], eb[:], op=OP.add)
            nc.vector.scalar_tensor_tensor(arg3, pc3, -GINV, m1003,
                                           op0=OP.mult, op1=OP.add)
            nc.scalar.activation(eb[:], arg[:], AF.Exp)
            nc.vector.tensor_tensor(ea[:], ea[:], eb[:], op=OP.add)  # ea <- S
            nc.scalar.activation(ea[:], ea[:], AF.Ln)                # ea <- lnS
            nc.vector.tensor_scalar(ea[:], ea[:], 0.0, 1.2,
                                    op0=OP.max, op1=OP.min)
            nc.vector.scalar_tensor_tensor(
                eb[:], ea[:], 1.0, s1[:], op0=OP.mult, op1=OP.mult,
                accum_out=packed[:, 1 + NCHUNK + c:2 + NCHUNK + c])
        # reduce over partitions on device (ones.T @ packed) so only 17
        # floats cross the tunnel per core
        ones = cpool.tile([128, 1], F32, tag="ones")
        nc.vector.memset(ones[:], 1.0)
        psum_s = ppool.tile([1, 1 + 2 * NCHUNK], F32, tag="psum_s")
        nc.tensor.matmul(psum_s[:], ones[:], packed[:],
                         start=True, stop=True)
        sums = cpool.tile([1, 1 + 2 * NCHUNK], F32, tag="sums")
        nc.vector.tensor_copy(sums[:], psum_s[:])
        nc.sync.dma_start(out_t.ap(), sums[:])

    _split_multiwaits(nc)
    return nc


def _make_runner(nc, n_cores):
    import jax
    from jax.sharding import Mesh, PartitionSpec
    from jax.experimental.shard_map import shard_map
    from concourse import bass2jax
    from concourse.bass2jax import _bass_exec_p, partition_id_tensor

    bass2jax.install_neuronx_cc_hook()

    partition_name = nc.partition_id_tensor.name if nc.partition_id_tensor else None
    in_names, out_names, out_avals = [], [], []
    for alloc in nc.m.functions[0].allocations:
        if not isinstance(alloc, mybir.MemoryLocationSet):
            continue
        name = alloc.memorylocations[0].name
        if alloc.kind == "ExternalInput":
            if name != partition_name:
                in_names.append(name)
        elif alloc.kind == "ExternalOutput":
            shape = tuple(alloc.tensor_shape)
            dtype = mybir.dt.np(alloc.dtype)
            out_names.append(name)
            out_avals.append(jax.core.ShapedArray(shape, dtype))
    n_params = len(in_names)
    # outputs are fully written by the kernel, so no zero-init buffers are
    # passed (each extra transferred array costs real tunnel latency)
    all_in_names = list(in_names)
    if partition_name is not None:
        all_in_names.append(partition_name)

    def _body(*args):
        operands = list(args)
        if partition_name is not None:
            operands.append(partition_id_tensor())
        outs = _bass_exec_p.bind(
            *operands,
            out_avals=tuple(out_avals),
            in_names=tuple(all_in_names),
            out_names=tuple(out_names),
            lowering_input_output_aliases=(),
            sim_require_finite=True,
            sim_require_nnan=True,
            nc=nc,
        )
        return tuple(outs)

    devices = jax.devices()[:n_cores]
    mesh = Mesh(np.asarray(devices), ("core",))
    in_specs = (PartitionSpec("core"),) * n_params
    out_specs = (PartitionSpec("core"),) * len(out_names)
    sm = shard_map(_body, mesh=mesh, in_specs=in_specs, out_specs=out_specs,
                   check_rep=False)
    in_shapes, in_dtypes = [], []
    for alloc in nc.m.functions[0].allocations:
        if (isinstance(alloc, mybir.MemoryLocationSet)
                and alloc.kind == "ExternalInput"
                and alloc.memorylocations[0].name in in_names):
            in_shapes.append(tuple(alloc.tensor_shape))
            in_dtypes.append(mybir.dt.np(alloc.dtype))
    example_args = [
        np.zeros((s[0] * n_cores,) + s[1:], d)
        for s, d in zip(in_shapes, in_dtypes)
    ]
    try:
        # C++ fast-path dispatch (~1ms less per-call python overhead)
        jitted = bass2jax.fast_dispatch_compile(
            lambda: jax.jit(sm, keep_unused=True)
            .lower(*example_args).compile())
    except Exception:
        jitted = jax.jit(sm, keep_unused=True)

    def run(in_maps):
        assert len(in_maps) == n_cores
        args = []
        for n in in_names:
            args.append(np.concatenate([np.asarray(m[n]) for m in in_maps], axis=0))
        outs = jitted(*args)
        results = [dict() for _ in range(n_cores)]
        for i, n in enumerate(out_names):
            full = np.asarray(outs[i])
            per = full.shape[0] // n_cores
            for cc in range(n_cores):
                results[cc][n] = full[cc * per:(cc + 1) * per]
        return results

    return run


def _get_runner():
    if not _RUNNER:
        nc = _build_module()
        _RUNNER.append(_make_runner(nc, NCORES))
    return _RUNNER[0]


def make_in_maps(pred, target):
    pt = np.empty((B, 2 * N), np.float16)  # one-pass pack + f16 convert
    pt[:, :N] = np.asarray(pred)[..., 0]
    pt[:, N:] = np.asarray(target)[..., 0]
    # row slices of the C-contiguous pack are already contiguous views
    return [{"pt": pt[c * BPC:(c + 1) * BPC]} for c in range(NCORES)]


def combine(results):
    vals_sum = 0.0
    acc_sum = 0.0
    corr_sum = 0.0
    for r in results:
        o = np.asarray(r["out"], dtype=np.float64)
        # col 0 holds the partition-sum of vals replicated GPB times
        vals_sum += float(o[0, 0]) / GPB
        acc_sum += float(np.sum(o[0, 1:1 + NCHUNK]))
        corr_sum += float(np.sum(o[0, 1 + NCHUNK:]))
    loss_shape = (vals_sum - GAMMA * corr_sum) / B
    loss_temporal = acc_sum / (B * N * N)
    return np.float32(ALPHA * loss_shape + (1.0 - ALPHA) * loss_temporal)


def _results_ok(results):
    for r in results:
        if not np.isfinite(r["out"]).all():
            return False
    return True


def kernel(pred, target):
    run = _get_runner()
    in_maps = make_in_maps(pred, target)
    out = None
    for attempt in range(3):
        try:
            results = run(in_maps)
        except Exception:
            # transient device errors (e.g. NRT exec-unit resets)
            if attempt == 2:
                raise
            import time as _time
            _time.sleep(2.0)
            continue
        if _results_ok(results):
            out = combine(results)
            break
        # silent bad execution (observed once after a device reset): retry
    else:
        out = combine(results)
    return out



# revision 25
# speedup vs baseline: 1.0222x; 1.0222x over previous
"""DILATE loss (soft-DTW shape + temporal distortion) Trainium2 Bass kernel.

Math (per batch element, N=256, gamma=0.01, alpha=0.8):
  D[i,j] = (t_i - p_j)^2
  soft-DTW DP: R[i,j] = D[i,j] + softmin_g(R[i-1,j-1], R[i-1,j], R[i,j-1])
  loss = alpha*mean_b R[N,N] + (1-alpha)*sum_ij mean_b(E)*(i-j)^2 / N^2,
  E = dR[N,N]/dD.

Kernel strategy:
  * gamma is tiny, so the hard-min DP is within ~5e-4 of the soft DP; each
    DP row is ONE raw tensor_tensor_scan(min,add) with interleaved APs
    (2 stream elements per cell: e1 mins the diagonal pred, e2 mins the
    up pred and adds D_j, written compactly via a step-0 output dim), with
    the D row produced on the scalar engine via Square(p + bias=-t_i).
    Forward and reverse DPs run together on 32 partitions per core.
  * E uses the forward/backward identity
      E[i,j] = exp((R[N,N] - Rf[i,j] - Rb[i,j] + D[i,j])/gamma)
    (Rb = DP of the axis-reversed cost matrix), fully elementwise.
  * first-order softness correction for the value:
      val_soft ~= val_hard - gamma * sum_ij E[i,j]*ln S[i,j],
      S[i,j] = sum_preds exp((minh[i,j] - Rh[pred])/gamma),
    which cuts total error another ~10x (to ~6e-5 relative).

Distribution: batch 128 -> 16 per core x 8 cores (data parallel; the
sharding_hint's all-reduce is replaced by a host-side combine of tiny
per-core partial sums).

I/O design (dominates wall time on this axon tunnel, where each call costs
~1 network round trip ~25-30ms plus ~70ms per EXTRA output array and a few
ms per extra input array):
  * ONE f16 input "pt" [16, 512] per core (p | t packed on the free axis),
    upcast to f32 on device (f16 quantization adds ~1e-5 rel err, budget
    is 2e-2).
  * ONE [1, 17] f32 output per core: partition-sums of (vals*8 | acc |
    acc2) computed on device by a ones-vector matmul, so only 68 bytes
    cross the tunnel per core.
  * no zero-init buffers are passed for outputs, and the jit uses the
    fast-dispatch (effect-free C++) path.
"""
import numpy as np
from contextlib import ExitStack

import bass_rust
import concourse.bass as bass
import concourse.mybir as mybir
import concourse.tile as tile

ALPHA = 0.8
GAMMA = 0.01
GINV = 1.0 / GAMMA
BIG = 1e8
B, N, NCORES = 128, 256, 8
BPC = B // NCORES          # 16 batches per core
P = 2 * BPC                # 32 scan partitions (fwd + bwd)
GPB = 128 // BPC           # 8 partition groups per batch in staged layout
RPG = N // GPB             # 32 rows per group
F32 = mybir.dt.float32
AF = mybir.ActivationFunctionType
OP = mybir.AluOpType
W = N + 1                  # row slot width (border col + N values)
# staged fwd region: 33 slots (1 overlap row + 32 rows) x 257 each
FOFF = 0
FSLOT = W
FSIZE = 33 * FSLOT
# staged bwd region: 32 slots x 256, natural element order
BOFF = FSIZE
BSIZE = RPG * N
NCHUNK = 8
SPC = RPG // NCHUNK        # 4 row-slots per E-pass chunk
FE = SPC * N               # 1024 free elems per chunk

_RUNNER = []


def _split_multiwaits(nc, max_waits=1):
    """This walrus build rejects any instruction carrying more than one
    semaphore wait ("Too many sync wait commands" at codegen); move excess
    waits onto preceding same-engine NoOps."""
    cnt = 0
    for f in nc.m.functions:
        for blk in f.blocks:
            newinsts = []
            changed = False
            for inst in blk.instructions:
                si = inst.sync_info
                if si is not None and si.on_wait is not None and len(si.on_wait) > max_waits:
                    waits = list(si.on_wait)
                    excess, keep = waits[:-max_waits], waits[-max_waits:]
                    while excess:
                        chunk, excess = excess[:max_waits], excess[max_waits:]
                        cnt += 1
                        newinsts.append(mybir.InstNoOp(
                            name=f"waitsplit{cnt}", engine=inst.engine,
                            ins=[], outs=[],
                            sync_info=mybir.SyncInfo(on_wait=chunk, on_update=[])))
                        changed = True
                    si.on_wait = keep
                newinsts.append(inst)
            if changed:
                blk.instructions[:] = newinsts


def _build_module():
    nc = bass.Bass()
    # single packed f16 input (p in cols 0:N, t in cols N:2N) and single
    # [1,17] output (col 0 = vals sum x GPB, 1:9 = acc col-sums, 9:17 =
    # acc2 col-sums, reduced over partitions on device). Array count and
    # payload bytes dominate per-call cost on this tunnel: each extra
    # output array is a serialized ~70ms round trip, and h2d/d2h move at
    # ~25 MB/s, so f16 input + 68B output shave several ms.
    F16 = mybir.dt.float16
    pt_in = nc.dram_tensor("pt", [BPC, 2 * N], F16, kind="ExternalInput")
    out_t = nc.dram_tensor("out", [1, 1 + 2 * NCHUNK], F32,
                           kind="ExternalOutput")

    with tile.TileContext(nc) as tc, ExitStack() as ctx:
        cpool = ctx.enter_context(tc.tile_pool(name="cpool", bufs=1))
        dpool = ctx.enter_context(tc.tile_pool(name="dpool", bufs=8))
        vpool = ctx.enter_context(tc.tile_pool(name="vpool", bufs=4))
        epool = ctx.enter_context(tc.tile_pool(name="epool", bufs=2))
        spool = ctx.enter_context(tc.tile_pool(name="spool", bufs=1))
        ppool = ctx.enter_context(tc.psum_pool(name="ppool", bufs=1))

        p_buf = cpool.tile([P, N], F32, tag="p_buf")
        t_buf = cpool.tile([P, N], F32, tag="t_buf")
        nt_buf = cpool.tile([P, N], F32, tag="nt_buf")
        tmp = cpool.tile([P, N], F32, tag="tmp")
        tstage = cpool.tile([128, RPG], F32, tag="tstage")
        pstage = cpool.tile([128, N], F32, tag="pstage")
        omega = cpool.tile([128, RPG * N], F32, tag="omega")
        g32 = cpool.tile([128, 1], F32, tag="g32")
        # g32[p] = RPG*(p%GPB): iota the 8 group offsets along the free dim,
        # then scatter to the interleaved partition layout with 8 tiny DMAs
        # (compute ops can't address stride-8 partitions; DMA can).
        giota = cpool.tile([BPC, GPB], F32, tag="giota")
        nc.gpsimd.iota(giota[:], pattern=[[RPG, GPB]], base=0,
                       channel_multiplier=0,
                       allow_small_or_imprecise_dtypes=True)
        g3v = g32.rearrange("(x y) f -> x y f", y=GPB)
        for g in range(GPB):
            nc.sync.dma_start(g3v[:, g, :].squeeze(), giota[:, g:g + 1])
        # on-device input prep: f16 DMA in, upcast to f32, fwd halves
        # straight, bwd halves reversed. compute ops must start at a
        # partition quadrant, so reverse at base 0 and DMA into the
        # upper half.
        pt16 = cpool.tile([BPC, 2 * N], F16, tag="pt16")
        nc.sync.dma_start(pt16[:], pt_in.ap())
        nc.vector.tensor_copy(p_buf[0:BPC, :], pt16[:, 0:N])
        nc.vector.tensor_copy(tmp[0:BPC, :], p_buf[0:BPC, ::-1])
        nc.sync.dma_start(p_buf[BPC:P, :], tmp[0:BPC, :])
        nc.vector.tensor_copy(t_buf[0:BPC, :], pt16[:, N:2 * N])
        nc.vector.tensor_copy(tmp[0:BPC, :], t_buf[0:BPC, ::-1])
        nc.sync.dma_start(t_buf[BPC:P, :], tmp[0:BPC, :])
        nc.vector.tensor_scalar_mul(nt_buf[:], t_buf[:], -1.0)
        # staged-layout replicas of p and t, from the upcast SBUF copies
        ts3 = tstage.rearrange("(x y) f -> x y f", y=GPB)
        ps3 = pstage.rearrange("(x y) f -> x y f", y=GPB)
        for g in range(GPB):
            nc.sync.dma_start(ts3[:, g, :].squeeze(),
                              t_buf[0:BPC, g * RPG:(g + 1) * RPG])
            nc.sync.dma_start(ps3[:, g, :].squeeze(), p_buf[0:BPC, :])
        # Omega[p, r*256+jm1] = ((32*(p%8) + r) - jm1)^2, built on device:
        # iota gives (r - jm1) per partition; Square adds the 32g bias.
        nc.gpsimd.iota(omega[:], pattern=[[1, RPG], [-1, N]], base=0,
                       channel_multiplier=0,
                       allow_small_or_imprecise_dtypes=True)
        nc.scalar.activation(omega[:], omega[:], AF.Square,
                             bias=g32[:], scale=1.0)

        stage = spool.tile([128, FSIZE + BSIZE], F32, tag="stage")
        # fwd region views: [x=16, y=8 groups, slot, elem]
        stF = stage[:, FOFF:FOFF + FSIZE].rearrange(
            "(x y) (s w) -> x y s w", y=GPB, w=FSLOT)
        stB = stage[:, BOFF:BOFF + BSIZE].rearrange(
            "(x y) (s w) -> x y s w", y=GPB, w=N)

        # rolling window: slot 0 = initial row [0, BIG...], 16 working slots
        win = cpool.tile([P, 17 * W], F32, tag="win")
        nc.vector.memset(win[:], BIG)
        nc.vector.memset(win[:, 0:1], 0.0)      # R[0,0] = 0
        winf = win[0:BPC].rearrange("p (s w) -> p s w", w=W)
        winb = win[BPC:P].rearrange("p (s w) -> p s w", w=W)

        # stage the fwd border row (row 0) into group 0's overlap slot
        nc.sync.dma_start(stF[:, 0, 0, :].squeeze(), win[0:BPC, 0:W])

        # pre-zero the 8 drow slots: evens stay 0 (the "+0" scan elements)
        for _z in range(8):
            zt = dpool.tile([P, 2 * N], F32, tag="drow")
            nc.vector.memset(zt[:], 0.0)

        V2 = bass_rust.VecI64Pair

        def _ap3(ap, d1, d2):
            part = tuple(ap.ap[0])
            ap.ap = V2([part, d1, d2])
            return ap

        prev_off = 0
        for i in range(1, N + 1):
            k = 1 + (i - 1) % 16
            off = k * W
            drow = dpool.tile([P, 2 * N], F32, tag="drow")
            nc.scalar.activation(drow[:, 1::2], p_buf[:], AF.Square,
                                 bias=nt_buf[:, i - 1:i], scale=1.0)
            # fused 3-way-min DP row: one scan, 2 stream elements per cell:
            #   e1: state = min(Rprev[j-1], state) + 0
            #   e2: state = min(Rprev[j],   state) + D_j
            d0 = _ap3(win[:, prev_off:prev_off + N], (1, N), (1, 2))
            d1 = _ap3(drow[:, 0:2 * N], (2, N), (1, 2))
            o3 = _ap3(win[:, off + 1:off + 1 + N], (1, N), (0, 2))
            eng = nc.vector
            eng.add_instruction(mybir.InstTensorScalarPtr(
                name=nc.get_next_instruction_name(),
                is_tensor_tensor_scan=True, is_scalar_tensor_tensor=True,
                op0=OP.min, op1=OP.add,
                ins=[eng.lower_ap(d0), eng.lower_ap_or_imm(float(BIG)),
                     eng.lower_ap(d1)],
                outs=[eng.lower_ap(o3)]))
            if i % 8 == 0:
                k0 = k - 7
                g, r = (i - 8) // RPG, (i - 8) % RPG
                # fwd rows staged full-width (border col included) at slots 1+r..
                nc.sync.dma_start(stF[:, g, 1 + r:1 + r + 8, :].squeeze(),
                                  winf[:, k0:k0 + 8, :])
                # bwd row i' lands at the (g,r) of fwd row 257-i', natural order
                gb, rb = (N - i) // RPG, (N - i) % RPG
                bstop = rb - 1 if rb > 0 else None
                nc.sync.dma_start(
                    stB[:, gb, rb + 7:bstop:-1, :].squeeze(),
                    winb[:, k0:k0 + 8, 1:W])
                if i % RPG == 0 and i < N:
                    # row i = 32g+32 is also group g+1's overlap slot 0
                    nc.sync.dma_start(stF[:, g + 1, 0, :].squeeze(),
                                      winf[:, k0 + 7, :])
            prev_off = off

        # per-batch DP value val_b = Rf[N,N]: group 7, slot 32, elem 256
        vcol16 = cpool.tile([BPC, 1], F32, tag="vcol16")
        nc.sync.dma_start(vcol16[:],
                          stF[:, GPB - 1, RPG, FSLOT - 1:FSLOT].squeeze())
        val128 = cpool.tile([128, 1], F32, tag="val128")
        v3 = val128.rearrange("(x y) f -> x y f", y=GPB)
        for g in range(GPB):
            nc.sync.dma_start(v3[:, g, :].squeeze(), vcol16[:])
        val100 = cpool.tile([128, 1], F32, tag="val100")
        nc.vector.tensor_scalar_mul(val100[:], val128[:], GINV)
        packed = cpool.tile([128, 1 + 2 * NCHUNK], F32, tag="packed")
        nc.vector.tensor_copy(packed[:, 0:1], val128[:])

        # E-pass over chunks of SPC row-slots:
        #   dq  = 100*D               (ACT Square(10*(t-p)))
        #   ex  = -100*(Rf+Rb) + dq   -> E = Exp(ex + 100*val)
        #   acc += E*Omega
        #   m100 = 100*Rf - dq (= 100*minh); S = sum_preds exp(m100 - 100*P)
        #   acc2 += E*clamp(ln S, 0, 1.2)
        # all-partition (128) views of the staged regions
        eF = stage[:, FOFF:FOFF + FSIZE].rearrange("p (s w) -> p s w", w=FSLOT)
        eB = stage[:, BOFF:BOFF + BSIZE].rearrange("p (s w) -> p s w", w=N)
        for c in range(NCHUNK):
            s0 = c * SPC                       # first row-slot (0-based row idx)
            rf3 = eF[:, 1 + s0:1 + s0 + SPC, 1:W]
            rb3 = eB[:, s0:s0 + SPC, ::-1]
            pa3 = eF[:, s0:s0 + SPC, 0:N]      # R[i-1, j-1]
            pb3 = eF[:, s0:s0 + SPC, 1:W]      # R[i-1, j]
            pc3 = eF[:, 1 + s0:1 + s0 + SPC, 0:N]  # R[i, j-1]

            def t3(tag):
                tl = epool.tile([128, FE], F32, tag=tag)
                return tl, tl.rearrange("p (s w) -> p s w", w=N)

            s1, s13 = t3("s1")
            nc.vector.tensor_tensor(s13, rf3, rb3, op=OP.add)
            u, u3 = t3("u")
            tb = tstage[:, s0:s0 + SPC].unsqueeze(2).broadcast_to((128, SPC, N))
            pb = pstage.unsqueeze(1).broadcast_to((128, SPC, N))
            nc.vector.tensor_tensor(u3, tb, pb, op=OP.subtract)
            nc.scalar.activation(u[:], u[:], AF.Square,
                                 bias=0.0, scale=GINV ** 0.5)  # u <- 100*D
            nc.vector.scalar_tensor_tensor(s1[:], s1[:], -GINV, u[:],
                                           op0=OP.mult, op1=OP.add)  # s1 <- ex
            nc.scalar.activation(s1[:], s1[:], AF.Exp,
                                 bias=val100[:], scale=1.0)          # s1 <- E
            # correction: m100 = 100*Rf - dq
            m100, m1003 = t3("m100")
            nc.vector.scalar_tensor_tensor(m1003, rf3, GINV, u3,
                                           op0=OP.mult, op1=OP.subtract)
            ea, ea3 = t3("ea")
            arg, arg3 = t3("arg")
            # temporal term: out goes to the (not yet used) arg scratch so E
            # in s1 survives for the correction below
            nc.vector.scalar_tensor_tensor(arg[:], s1[:], 1.0,
                                           omega[:, c * FE:(c + 1) * FE],
                                           op0=OP.mult, op1=OP.mult,
                                           accum_out=packed[:, 1 + c:2 + c])
            nc.vector.scalar_tensor_tensor(arg3, pa3, -GINV, m1003,
                                           op0=OP.mult, op1=OP.add)
            nc.scalar.activation(ea[:], arg[:], AF.Exp)
            eb, eb3 = t3("eb")
            nc.vector.scalar_tensor_tensor(arg3, pb3, -GINV, m1003,
                                           op0=OP.mult, op1=OP.add)
            nc.scalar.activation(eb[:], arg[:], AF.Exp)
            nc.vector.tensor_tensor(ea[:], ea[:], eb[:], op=OP.add)
            nc.vector.scalar_tensor_tensor(arg3, pc3, -GINV, m1003,
                                           op0=OP.mult, op1=OP.add)
            nc.scalar.activation(eb[:], arg[:], AF.Exp)
            nc.vector.tensor_tensor(ea[:], ea[:], eb[:], op=OP.add)  # ea <- S
            nc.scalar.activation(ea[:], ea[:], AF.Ln)                # ea <- lnS
            nc.vector.tensor_scalar(ea[:], ea[:], 0.0, 1.2,
                                    op0=OP.max, op1=OP.min)
            nc.vector.scalar_tensor_tensor(
                eb[:], ea[:], 1.0, s1[:], op0=OP.mult, op1=OP.mult,
                accum_out=packed[:, 1 + NCHUNK + c:2 + NCHUNK + c])
        # reduce over partitions on device (ones.T @ packed) so only 17
        # floats cross the tunnel per core
        ones = cpool.tile([128, 1], F32, tag="ones")
        nc.vector.memset(ones[:], 1.0)
        psum_s = ppool.tile([1, 1 + 2 * NCHUNK], F32, tag="psum_s")
        nc.tensor.matmul(psum_s[:], ones[:], packed[:],
                         start=True, stop=True)
        sums = cpool.tile([1, 1 + 2 * NCHUNK], F32, tag="sums")
        nc.vector.tensor_copy(sums[:], psum_s[:])
        nc.sync.dma_start(out_t.ap(), sums[:])

    _split_multiwaits(nc)
    return nc


def _make_runner(nc, n_cores):
    import jax
    from jax.sharding import Mesh, PartitionSpec
    from jax.experimental.shard_map import shard_map
    from concourse import bass2jax
    from concourse.bass2jax import _bass_exec_p, partition_id_tensor

    bass2jax.install_neuronx_cc_hook()

    partition_name = nc.partition_id_tensor.name if nc.partition_id_tensor else None
    in_names, out_names, out_avals = [], [], []
    for alloc in nc.m.functions[0].allocations:
        if not isinstance(alloc, mybir.MemoryLocationSet):
            continue
        name = alloc.memorylocations[0].name
        if alloc.kind == "ExternalInput":
            if name != partition_name:
                in_names.append(name)
        elif alloc.kind == "ExternalOutput":
            shape = tuple(alloc.tensor_shape)
            dtype = mybir.dt.np(alloc.dtype)
            out_names.append(name)
            out_avals.append(jax.core.ShapedArray(shape, dtype))
    n_params = len(in_names)
    # outputs are fully written by the kernel, so no zero-init buffers are
    # passed (each extra transferred array costs real tunnel latency)
    all_in_names = list(in_names)
    if partition_name is not None:
        all_in_names.append(partition_name)

    def _body(*args):
        operands = list(args)
        if partition_name is not None:
            operands.append(partition_id_tensor())
        outs = _bass_exec_p.bind(
            *operands,
            out_avals=tuple(out_avals),
            in_names=tuple(all_in_names),
            out_names=tuple(out_names),
            lowering_input_output_aliases=(),
            sim_require_finite=True,
            sim_require_nnan=True,
            nc=nc,
        )
        return tuple(outs)

    devices = jax.devices()[:n_cores]
    mesh = Mesh(np.asarray(devices), ("core",))
    in_specs = (PartitionSpec("core"),) * n_params
    out_specs = (PartitionSpec("core"),) * len(out_names)
    sm = shard_map(_body, mesh=mesh, in_specs=in_specs, out_specs=out_specs,
                   check_rep=False)
    in_shapes, in_dtypes = [], []
    for alloc in nc.m.functions[0].allocations:
        if (isinstance(alloc, mybir.MemoryLocationSet)
                and alloc.kind == "ExternalInput"
                and alloc.memorylocations[0].name in in_names):
            in_shapes.append(tuple(alloc.tensor_shape))
            in_dtypes.append(mybir.dt.np(alloc.dtype))
    example_args = [
        np.zeros((s[0] * n_cores,) + s[1:], d)
        for s, d in zip(in_shapes, in_dtypes)
    ]
    try:
        # C++ fast-path dispatch (~1ms less per-call python overhead)
        jitted = bass2jax.fast_dispatch_compile(
            lambda: jax.jit(sm, keep_unused=True)
            .lower(*example_args).compile())
    except Exception:
        jitted = jax.jit(sm, keep_unused=True)
    # warm the executable (NEFF device-load + PJRT paths) off the timed path
    for _ in range(2):
        try:
            for o in jitted(*example_args):
                np.asarray(o)
        except Exception:
            break

    def run(in_maps):
        assert len(in_maps) == n_cores
        args = []
        for n in in_names:
            args.append(np.concatenate([np.asarray(m[n]) for m in in_maps], axis=0))
        outs = jitted(*args)
        results = [dict() for _ in range(n_cores)]
        for i, n in enumerate(out_names):
            full = np.asarray(outs[i])
            per = full.shape[0] // n_cores
            for cc in range(n_cores):
                results[cc][n] = full[cc * per:(cc + 1) * per]
        return results

    return run


def _get_runner():
    if not _RUNNER:
        nc = _build_module()
        _RUNNER.append(_make_runner(nc, NCORES))
    return _RUNNER[0]


def make_in_maps(pred, target):
    pt = np.empty((B, 2 * N), np.float16)  # one-pass pack + f16 convert
    pt[:, :N] = np.asarray(pred)[..., 0]
    pt[:, N:] = np.asarray(target)[..., 0]
    # row slices of the C-contiguous pack are already contiguous views
    return [{"pt": pt[c * BPC:(c + 1) * BPC]} for c in range(NCORES)]


def combine(results):
    vals_sum = 0.0
    acc_sum = 0.0
    corr_sum = 0.0
    for r in results:
        o = np.asarray(r["out"], dtype=np.float64)
        # col 0 holds the partition-sum of vals replicated GPB times
        vals_sum += float(o[0, 0]) / GPB
        acc_sum += float(np.sum(o[0, 1:1 + NCHUNK]))
        corr_sum += float(np.sum(o[0, 1 + NCHUNK:]))
    loss_shape = (vals_sum - GAMMA * corr_sum) / B
    loss_temporal = acc_sum / (B * N * N)
    return np.float32(ALPHA * loss_shape + (1.0 - ALPHA) * loss_temporal)


def _results_ok(results):
    for r in results:
        if not np.isfinite(r["out"]).all():
            return False
    return True


def kernel(pred, target):
    run = _get_runner()
    in_maps = make_in_maps(pred, target)
    out = None
    for attempt in range(3):
        try:
            results = run(in_maps)
        except Exception:
            # transient device errors (e.g. NRT exec-unit resets)
            if attempt == 2:
                raise
            import time as _time
            _time.sleep(2.0)
            continue
        if _results_ok(results):
            out = combine(results)
            break
        # silent bad execution (observed once after a device reset): retry
    else:
        out = combine(results)
    return out

